# revision 1
# baseline (speedup 1.0000x reference)
"""ChainGNN Trainium2 kernel (8-core SPMD, node-sharded).

Reference math:
  enc = relu(x@W1+b1) @ W2 + b2            -> (B, N, GH)
  2x GCN layers: out += A_hat @ (LN(out) @ Wg)
  y = (out @ Wout + bout).squeeze(-1)      -> (B, N)

Exploited structure (host-verified at runtime):
  * A_hat = D^-1/2 A D^-1/2 for a 0/1 chain adjacency: A_hat[n,m] = a[n]a[m]
    for |n-m|==1, zero diagonal. The graph mix becomes shifted adds.
  * Layer 2 collapses to row functionals of out1 (see tail assembly).
  * W2 columns are pre-divided by a[n] on the host, so the device works in
    X' = enc/a coordinates: xc = (X'-mu')*(a/sigma') equals a*LN(enc) and
    out1' = out1/a = X' + u_l + u_r needs NO broadcast scaling on big data.
    All a-factors live in small [128, NH] per-node tensors in the tail.
  * Nodes sharded 8 ways: 256 owned/core, 260-window (2-halo). Window split
    into two 132-node halves packed on partitions p = half*64 + batch.
  * Per-node LN eps becomes eps/a^2, added as a small tensor before sqrt.
  * Variance row-sums use a bf16 pairwise tree (DVE 2x rate) + f32 tail
    reduce instead of a full-rate f32 tensor_reduce.

Device layout: X[p, n, g] bf16; per-(p,n) scalars live as (128,NH) f32
tensors; the chain mix is free-dim shifts.
"""

import sys

sys.path.insert(0, "/opt/trn_rl_repo")

from contextlib import ExitStack

import ml_dtypes
import numpy as np

import concourse.bacc as bacc
import concourse.bass as bass  # noqa: F401
import concourse.mybir as mybir
import concourse.tile as tile
from concourse import masks

F32 = mybir.dt.float32
BF16 = mybir.dt.bfloat16
ALU = mybir.AluOpType
ACTF = mybir.ActivationFunctionType

B, ENC_IN, ENC_HID, N, GH = 64, 64, 256, 2048, 128
LN_EPS = 1e-5
NCORES = 8
OWN = N // NCORES
WIN = OWN + 4
NH = 132                     # nodes per half
EXT = NH + 2
RC = GH + 6                  # [Wg1' | eW | eV | fW | fV | f1 | pad]
NCOL = GH + 5
BLK = 24
CH = 2048                    # w2 stream chunk cols (bf16: 512KB DMAs)
G3 = 3                       # nodes per transpose/u-mm group
SQ128 = float(np.sqrt(128.0))
TREE = 2                     # bf16 pairwise-halving stages before f32 reduce
CMB = 12                     # combine chunk nodes

_PROGRAM_CACHE = {}

BLOCKS = []
_n0 = 0
while _n0 < NH:
    BLOCKS.append((_n0, min(_n0 + BLK, NH)))
    _n0 = BLOCKS[-1][1]


def build_program(reps=1):
    nc = bacc.Bacc()

    w2s = nc.dram_tensor("w2s", [128, 4 * NH * GH], BF16, kind="ExternalInput")
    w2ns = nc.dram_tensor("w2ns", [128, 4 * NH], BF16, kind="ExternalInput")
    xT = nc.dram_tensor("xT", [ENC_IN, B], F32, kind="ExternalInput")
    w1 = nc.dram_tensor("w1", [ENC_IN, ENC_HID], F32, kind="ExternalInput")
    b1r = nc.dram_tensor("b1r", [128, 2], F32, kind="ExternalInput")
    rhs1 = nc.dram_tensor("rhs1", [GH, RC], BF16, kind="ExternalInput")
    cst = nc.dram_tensor("cst", [128, 6 * NH], F32, kind="ExternalInput")
    y_out = nc.dram_tensor("y", [128, NH], F32, kind="ExternalOutput")

    with tile.TileContext(nc) as tc, ExitStack() as ctx:
        cp = ctx.enter_context(tc.tile_pool(name="const", bufs=1))
        xp = ctx.enter_context(tc.tile_pool(name="xdata", bufs=1))
        wp = ctx.enter_context(tc.tile_pool(name="w2buf", bufs=3))
        sp = ctx.enter_context(tc.tile_pool(name="smalls", bufs=1))
        xtp = ctx.enter_context(tc.tile_pool(name="xt", bufs=4))
        sqp = ctx.enter_context(tc.tile_pool(name="sq", bufs=3))
        scp = ctx.enter_context(tc.tile_pool(name="scomb", bufs=3))
        pse = ctx.enter_context(tc.tile_pool(name="psenc", bufs=3, space="PSUM"))
        pst = ctx.enter_context(tc.tile_pool(name="pstr", bufs=2, space="PSUM"))
        psu = ctx.enter_context(tc.tile_pool(name="psu", bufs=2, space="PSUM"))
        psm = ctx.enter_context(tc.tile_pool(name="psmisc", bufs=1, space="PSUM"))

        # ---- constants ----
        ident = cp.tile([128, 128], BF16, tag="ident", name="ident")
        masks.make_identity(nc, ident[:])

        def const(handle, shape, tag, dt=F32):
            t = cp.tile(shape, dt, tag=tag, name=tag)
            nc.sync.dma_start(t[:], handle[:])
            return t

        c_xT = const(xT, [ENC_IN, B], "c_xT")
        c_w1 = const(w1, [ENC_IN, ENC_HID], "c_w1")
        c_b1 = const(b1r, [128, 2], "c_b1")
        c_rhs1 = const(rhs1, [GH, RC], "c_rhs1", dt=BF16)
        c_cst = const(cst, [128, 6 * NH], "c_cst")
        c_a = c_cst[:, 0 * NH:1 * NH]
        c_a128 = c_cst[:, 1 * NH:2 * NH]
        c_asw = c_cst[:, 2 * NH:3 * NH]
        c_asv = c_cst[:, 3 * NH:4 * NH]
        c_eps = c_cst[:, 4 * NH:5 * NH]
        c_c2vec = c_cst[:, 5 * NH:6 * NH]
        c_w2ns = const(w2ns, [128, 4 * NH], "c_w2ns", dt=BF16)

        # ---- persistent tensors ----
        Xb = [xp.tile([128, (n1 - n0) * GH], BF16, tag=f"X{j}", name=f"X{j}")
              for j, (n0, n1) in enumerate(BLOCKS)]
        xj3 = [t[:].rearrange("p (n g) -> p n g", g=GH) for t in Xb]
        UG = xp.tile([128, EXT * RC], BF16, tag="UG", name="UG")
        ug3 = UG[:].rearrange("p (e c) -> p e c", c=RC)
        nc.vector.memset(ug3[:, 0:1, :], 0.0)
        nc.vector.memset(ug3[:, EXT - 1:EXT, :], 0.0)

        def small(tag):
            return sp.tile([128, NH], F32, tag=tag, name=tag)

        sum1 = small("sum1")      # 128*mu1'
        sqr1 = small("sqr1")      # sum_g X'^2
        negmu = small("negmu")    # -mu1'
        sq1 = small("sq1")        # sqrt(128*var1' + 128*eps/a^2)
        rs1a = small("rs1a")      # a / sigma1'eps
        sum2 = small("sum2")
        sqr2 = small("sqr2")
        sq2 = small("sq2")
        rc2 = small("rc2")        # sqrt(128)/sq2 = 1/sigma2'eps
        tpp = sp.tile([128, NH + 2], F32, tag="tpp", name="tpp")
        nc.vector.memset(tpp[:, 0:1], 0.0)
        nc.vector.memset(tpp[:, NH + 1:NH + 2], 0.0)

        for _rep in range(reps):
            # ---- h1 = relu(x@W1+b1), transposed, two k-chunks ----
            h1T = cp.tile([128, 2 * B], BF16, tag="h1T", name="h1T")
            for c in range(2):
                ph = psm.tile([128, 128], F32, tag="psmisc", name="psmisc")
                nc.tensor.matmul(ph[:, 0:B], c_w1[:, c * 128:(c + 1) * 128], c_xT[:],
                                 start=True, stop=True)
                nc.scalar.activation(h1T[:, c * B:(c + 1) * B], ph[:, 0:B], ACTF.Relu,
                                     bias=c_b1[:, c:c + 1], scale=1.0)

            # ---- row sums of X' via W2/a block-sums: sum1 = h1 @ w2ns ----
            psum1 = psm.tile([128, 256], F32, tag="psmisc", name="psmisc")
            for h in range(2):
                nc.tensor.matmul(psum1[h * 64:(h + 1) * 64, 0:NH], h1T[:, 0:B],
                                 c_w2ns[:, h * NH:(h + 1) * NH], start=True, stop=False,
                                 tile_position=(0, h * 64))
                nc.tensor.matmul(psum1[h * 64:(h + 1) * 64, 0:NH], h1T[:, B:2 * B],
                                 c_w2ns[:, (2 + h) * NH:(3 + h) * NH], start=False, stop=True,
                                 tile_position=(0, h * 64))
            nc.scalar.activation(sum1[:], psum1[:, 0:NH], ACTF.Copy)
            nc.vector.tensor_scalar(negmu[:], sum1[:], -1.0 / 128.0, None, ALU.mult)

            def stats_smalls(n0, n1, sum_d, sqr_d, sq_d, rsc_d, l2):
                """var128 = sqr - sum^2/128 + 128eps/a^2; sq = sqrt(.);
                rsc = a*sqrt(128)/sq (l1) or sqrt(128)/sq (l2)."""
                s_ = np.s_[:, n0:n1]
                w = n1 - n0
                t1 = sp.tile([128, BLK], F32, tag="t1", name="t1")
                nc.vector.tensor_tensor(t1[:, :w], sum_d[s_], sum_d[s_], ALU.mult)
                nc.vector.scalar_tensor_tensor(t1[:, :w], t1[:, :w], -1.0 / 128.0,
                                               sqr_d[s_], ALU.mult, ALU.add)
                nc.vector.tensor_tensor(t1[:, :w], t1[:, :w], c_eps[s_], ALU.add)
                nc.scalar.activation(sq_d[s_], t1[:, :w], ACTF.Sqrt,
                                     bias=0.0, scale=1.0)
                nc.vector.reciprocal(rsc_d[s_], sq_d[s_])
                if l2:
                    nc.vector.tensor_scalar(rsc_d[s_], rsc_d[s_], SQ128, None,
                                            ALU.mult)
                else:
                    nc.vector.tensor_tensor(rsc_d[s_], rsc_d[s_], c_a128[s_],
                                            ALU.mult)

            def sq_reduce(dst, src3, n0, n1, sq_eng="act"):
                """dst[:, n0:n1] = sum_g src3[:, :, g]^2 via bf16 tree.

                src3 is a [128, w, GH] bf16 view (w = n1-n0); uses a scratch
                tile; final f32 tensor_reduce over the last GH>>TREE cols.
                The initial square runs on sq_eng; tree+reduce on DVE.
                """
                w = n1 - n0
                t = sqp.tile([128, BLK * GH], BF16, tag="tree", name="tree")
                t3 = t[:, 0:w * GH].rearrange("p (n g) -> p n g", g=GH)
                if sq_eng == "act":
                    nc.scalar.activation(t3, src3, ACTF.Square)
                elif sq_eng == "pool":
                    nc.gpsimd.tensor_tensor(t3, src3, src3, ALU.mult)
                else:
                    nc.vector.tensor_tensor(t3, src3, src3, ALU.mult)
                hw = GH
                for _ in range(TREE):
                    hw //= 2
                    nc.vector.tensor_tensor(
                        t3[:, :, 0:hw], t3[:, :, 0:hw], t3[:, :, hw:2 * hw],
                        ALU.add)
                nc.vector.tensor_reduce(
                    dst[:, n0:n1], t3[:, :, 0:hw],
                    axis=mybir.AxisListType.X, op=ALU.add)

            def node_group(j, n0, g0, g1):
                """xc = (X'-mu')*rs1a -> PE transpose -> u-mm -> evac to UG."""
                ng = g1 - g0
                xc = xtp.tile([128, G3 * 128], BF16, tag="xc", name="xc")
                for m in range(g0, g1):
                    r = m - g0
                    nc.vector.tensor_scalar(xc[:, r * 128:(r + 1) * 128],
                                            xj3[j][:, m - n0, :],
                                            negmu[:, m:m + 1], rs1a[:, m:m + 1],
                                            ALU.add, op1=ALU.mult)
                p4 = pst.tile([128, G3 * 128], BF16, tag="p4", name="p4")
                for r in range(ng):
                    nc.tensor.transpose(p4[:, r * 128:(r + 1) * 128],
                                        xc[:, r * 128:(r + 1) * 128], ident[:])
                xt = xtp.tile([128, G3 * 128], BF16, tag="xt", name="xt")
                nc.vector.tensor_copy(xt[:, 0:ng * 128], p4[:, 0:ng * 128])
                pu = psu.tile([128, 512], F32, tag="pu", name="pu")
                for r in range(ng):
                    nc.tensor.matmul(pu[:, r * RC:r * RC + RC],
                                     xt[:, r * 128:(r + 1) * 128],
                                     c_rhs1[:], start=True, stop=True)
                # UG slots for nodes g0..g1 are contiguous: one evacuation
                nc.scalar.activation(UG[:, (g0 + 1) * RC:(g1 + 1) * RC],
                                     pu[:, 0:ng * RC], ACTF.Copy)

            def emit_combine(j, s0, s1, _demote=True):
                """X'[s0:s1) += u_l+u_r; then layer-2 sq-sums."""
                if _demote:
                    # deprioritize: background work fills engine idle slots
                    with tc.high_priority(offset=-200000):
                        return emit_combine(j, s0, s1, _demote=False)
                n0 = BLOCKS[j][0]
                w = s1 - s0
                S = scp.tile([128, CMB * GH], BF16, tag="Scomb", name="Scomb")
                s3 = S[:, 0:w * GH].rearrange("p (n g) -> p n g", g=GH)
                nc.vector.tensor_tensor(s3, ug3[:, s0:s0 + w, 0:GH],
                                        ug3[:, s0 + 2:s0 + 2 + w, 0:GH],
                                        ALU.add)
                o0 = (s0 - n0) * GH
                nc.vector.tensor_tensor(Xb[j][:, o0:o0 + w * GH],
                                        Xb[j][:, o0:o0 + w * GH],
                                        S[:, 0:w * GH], ALU.add)
                sq_reduce(sqr2, xj3[j][:, s0 - n0:s1 - n0, :], s0, s1,
                          sq_eng="pool" if s1 <= NH - 3 * CMB else "dve")

            combine_chunks = [(j, s0, min(s0 + CMB, n1))
                              for j, (n0, n1) in enumerate(BLOCKS)
                              for s0 in range(n0, n1, CMB)]
            combine_ptr = [0]

            def emit_safe_combines(evac_upto):
                # chunk (j, s0, s1) reads UG slots up to s1+1 (node s1);
                # safe once nodes < evac_upto are evacuated, i.e. s1 < evac_upto
                while combine_ptr[0] < len(combine_chunks):
                    j, s0, s1 = combine_chunks[combine_ptr[0]]
                    if s1 >= evac_upto:
                        break
                    emit_combine(j, s0, s1)
                    combine_ptr[0] += 1

            # ---- streaming enc ----
            banks_done = 0
            blk_emitted = 0
            for c0 in range(0, NH * GH, CH):
                ch = min(CH, NH * GH - c0)
                wtile = wp.tile([128, 4 * CH], BF16, tag="w2", name="w2")
                nc.sync.dma_start(wtile[:, 0:4 * ch],
                                  w2s[:, 4 * c0:4 * c0 + 4 * ch])

                def wt(kc, h, bs, _t=wtile, _ch=ch):
                    i = kc * 2 + h
                    return _t[:, i * _ch + bs:i * _ch + bs + 512]

                for bs in range(0, ch, 512):
                    pe = pse.tile([128, 512], F32, tag="pe", name="pe")
                    for h in range(2):
                        nc.tensor.matmul(pe[h * 64:(h + 1) * 64, :],
                                         h1T[:, 0:B], wt(0, h, bs),
                                         start=True, stop=False,
                                         tile_position=(0, h * 64))
                        nc.tensor.matmul(pe[h * 64:(h + 1) * 64, :],
                                         h1T[:, B:2 * B], wt(1, h, bs),
                                         start=False, stop=True,
                                         tile_position=(0, h * 64))
                    node0 = banks_done * 4
                    j = next(i for i, (a0, a1) in enumerate(BLOCKS)
                             if a0 <= node0 < a1)
                    bn0 = BLOCKS[j][0]
                    off = (node0 - bn0) * GH
                    nc.scalar.activation(Xb[j][:, off:off + 512], pe[:, :],
                                         ACTF.Copy)
                    banks_done += 1
                    while blk_emitted < len(BLOCKS) and \
                            BLOCKS[blk_emitted][1] <= banks_done * 4:
                        jj = blk_emitted
                        n0, n1 = BLOCKS[jj]
                        # sum of squares for LN1 (bf16 tree on DVE)
                        sq_reduce(sqr1, xj3[jj][:, 0:n1 - n0, :], n0, n1)
                        stats_smalls(n0, n1, sum1, sqr1, sq1, rs1a, False)
                        for g0 in range(n0, n1, G3):
                            node_group(jj, n0, g0, min(g0 + G3, n1))
                        blk_emitted += 1
                        emit_safe_combines(n1)

            emit_safe_combines(NH + 1)

            # ---- layer-2 smalls + y ----
            F1l = ug3[:, 0:NH, GH + 4:GH + 5]
            F1r = ug3[:, 2:NH + 2, GH + 4:GH + 5]
            s23 = sum2[:].rearrange("p (n o) -> p n o", o=1)
            nc.vector.tensor_tensor(s23, F1l, F1r, ALU.add)
            nc.vector.tensor_tensor(sum2[:], sum2[:], sum1[:], ALU.add)
            for j, (n0, n1) in enumerate(BLOCKS):
                stats_smalls(n0, n1, sum2, sqr2, sq2, rc2, True)

            EWp = ug3[:, 1:NH + 1, GH:GH + 1]
            EVp = ug3[:, 1:NH + 1, GH + 1:GH + 2]
            TWl, TWr = ug3[:, 0:NH, GH + 2:GH + 3], ug3[:, 2:NH + 2, GH + 2:GH + 3]
            TVl, TVr = ug3[:, 0:NH, GH + 3:GH + 4], ug3[:, 2:NH + 2, GH + 3:GH + 4]

            invrs = small("invrs")      # sigma1'eps = sq1/sqrt(128)
            nc.vector.tensor_scalar(invrs[:], sq1[:], 1.0 / SQ128, None, ALU.mult)
            iv3 = invrs[:].rearrange("p (n o) -> p n o", o=1)
            tmps = small("tmps")
            xew = small("xew")
            nc.vector.tensor_tensor(xew[:].rearrange("p (n o) -> p n o", o=1),
                                    EWp, iv3, ALU.mult)
            nc.vector.tensor_tensor(tmps[:], sum1[:], c_asw[:], ALU.mult)
            nc.vector.tensor_tensor(xew[:], xew[:], tmps[:], ALU.add)
            xev = small("xev")
            nc.vector.tensor_tensor(xev[:].rearrange("p (n o) -> p n o", o=1),
                                    EVp, iv3, ALU.mult)
            nc.vector.tensor_tensor(tmps[:], sum1[:], c_asv[:], ALU.mult)
            nc.vector.tensor_tensor(xev[:], xev[:], tmps[:], ALU.add)
            d1 = small("d1")
            nc.vector.tensor_tensor(d1[:].rearrange("p (n o) -> p n o", o=1),
                                    TWl, TWr, ALU.add)
            nc.vector.tensor_tensor(d1[:], d1[:], c_a[:], ALU.mult)
            nc.vector.tensor_tensor(d1[:], d1[:], xew[:], ALU.add)
            d2 = small("d2")
            nc.vector.tensor_tensor(d2[:].rearrange("p (n o) -> p n o", o=1),
                                    TVl, TVr, ALU.add)
            nc.vector.tensor_tensor(d2[:], d2[:], c_a[:], ALU.mult)
            nc.vector.tensor_tensor(d2[:], d2[:], xev[:], ALU.add)
            tmp = small("tmp")
            nc.vector.tensor_tensor(tmp[:], sum2[:], c_asv[:], ALU.mult)
            nc.vector.tensor_tensor(tmp[:], d2[:], tmp[:], ALU.subtract)
            nc.vector.tensor_tensor(tpp[:, 1:NH + 1], tmp[:], rc2[:], ALU.mult)
            ys = small("ys")
            nc.vector.tensor_tensor(ys[:], tpp[:, 0:NH], tpp[:, 2:NH + 2], ALU.add)
            nc.vector.tensor_tensor(ys[:], ys[:], c_a[:], ALU.mult)
            nc.vector.tensor_tensor(d1[:], d1[:], c_c2vec[:], ALU.add)
            yt = small("yt")
            nc.vector.tensor_tensor(yt[:], ys[:], d1[:], ALU.add)
            nc.sync.dma_start(y_out[:], yt[:])

    nc.compile()
    return nc


def _host_factorize_ahat(A_hat):
    A = np.asarray(A_hat, np.float64)
    mask = A != 0
    np.fill_diagonal(mask, False)
    deg = mask.sum(1)
    a = 1.0 / np.sqrt(deg + 1e-8)
    recon = a[:, None] * mask * a[None, :]
    assert np.allclose(recon, A, atol=1e-5), "A_hat not normalized 0/1 adjacency"
    nz = np.argwhere(mask)
    assert np.all(np.abs(nz[:, 0] - nz[:, 1]) == 1), "A_hat not a chain"
    assert np.allclose(np.diag(A), 0.0)
    return a.astype(np.float32)


def host_prep(x, A_hat, W1, b1, W2, b2, Wg1, Wg2, ln_g1, ln_b1, ln_g2, ln_b2,
              Wout, bout):
    if np.any(np.asarray(b2) != 0):
        raise NotImplementedError("b2 != 0 unsupported")
    if np.any(np.asarray(ln_b1) != 0):
        raise NotImplementedError("ln_b1 != 0 unsupported")
    a_g = _host_factorize_ahat(A_hat)

    Wg1p = (np.asarray(ln_g1)[:, None] * np.asarray(Wg1)).astype(np.float32)
    eW = np.asarray(Wout)[:, 0].astype(np.float32)
    v2 = (np.asarray(Wg2) @ eW).astype(np.float32)
    eV = (np.asarray(ln_g2) * v2).astype(np.float32)
    fW = (Wg1p @ eW).astype(np.float32)
    fV = (Wg1p @ eV).astype(np.float32)
    f1 = Wg1p.sum(1).astype(np.float32)
    pad = np.zeros((GH, RC - NCOL), np.float32)
    rhs1 = np.concatenate(
        [Wg1p, eW[:, None], eV[:, None], fW[:, None], fV[:, None], f1[:, None],
         pad], 1).astype(ml_dtypes.bfloat16)
    seW, seV = float(eW.sum()), float(eV.sum())
    C2 = float(np.asarray(ln_b2) @ v2)
    bout_f = float(np.asarray(bout).reshape(-1)[0])

    xTh = np.ascontiguousarray(np.asarray(x).T, dtype=np.float32)
    b1r = np.ascontiguousarray(np.asarray(b1).reshape(2, 128).T,
                               dtype=np.float32)

    a_nb = np.zeros(N, np.float32)
    a_nb[:-1] += a_g[1:]
    a_nb[1:] += a_g[:-1]

    # X' = enc / a: divide W2 columns by a_n before casting to bf16
    W2r = (np.asarray(W2).reshape(ENC_HID, N, GH)
           / a_g[None, :, None]).astype(ml_dtypes.bfloat16)
    W2n = W2r.astype(np.float32).sum(-1).astype(ml_dtypes.bfloat16)  # (256, N)
    in_maps, starts = [], []
    for k in range(NCORES):
        start = min(max(OWN * k - 2, 0), N - WIN)
        starts.append(start)
        h0 = start + np.arange(NH)
        h1 = start + 128 + np.arange(NH)
        w2f = np.empty((4, 128, NH * GH), ml_dtypes.bfloat16)
        w2nsk = np.empty((4, 128, NH), ml_dtypes.bfloat16)
        for kc in range(2):
            rows = slice(kc * 128, (kc + 1) * 128)
            w2f[kc * 2 + 0] = W2r[rows][:, h0, :].reshape(128, NH * GH)
            w2f[kc * 2 + 1] = W2r[rows][:, h1, :].reshape(128, NH * GH)
            w2nsk[kc * 2 + 0] = W2n[rows][:, h0]
            w2nsk[kc * 2 + 1] = W2n[rows][:, h1]
        # chunk-interleave the 4 streams: [c0*4 : c0*4+4*ch] covers all 4
        w2sk = np.empty((128, 4 * NH * GH), ml_dtypes.bfloat16)
        for c0 in range(0, NH * GH, CH):
            ch = min(CH, NH * GH - c0)
            for i in range(4):
                w2sk[:, 4 * c0 + i * ch:4 * c0 + (i + 1) * ch] = \
                    w2f[i][:, c0:c0 + ch]
        w2nsk = np.ascontiguousarray(
            w2nsk.transpose(1, 0, 2).reshape(128, 4 * NH))
        a_tk = np.empty((128, NH), np.float32)
        a_tk[0:64] = a_g[h0][None, :]
        a_tk[64:128] = a_g[h1][None, :]
        c2 = np.empty((128, NH), np.float32)
        c2[0:64] = (C2 * a_g[h0] * a_nb[h0] + bout_f)[None, :]
        c2[64:128] = (C2 * a_g[h1] * a_nb[h1] + bout_f)[None, :]
        cstk = np.concatenate([
            a_tk, (a_tk * SQ128), (a_tk * (seW / 128.0)),
            (a_tk * (seV / 128.0)), (128.0 * LN_EPS / (a_tk * a_tk)), c2,
        ], axis=1).astype(np.float32)
        in_maps.append({
            "w2s": w2sk, "w2ns": w2nsk, "xT": xTh,
            "w1": np.ascontiguousarray(W1, np.float32), "b1r": b1r,
            "rhs1": rhs1, "cst": cstk,
        })
    return in_maps, starts


def assemble(results, starts):
    y = np.empty((B, N), np.float32)
    for k in range(NCORES):
        yk = np.asarray(results[k]["y"])
        start = starts[k]
        g = np.arange(OWN * k, OWN * (k + 1))
        w = g - start
        half = (w >= 130).astype(np.int64)
        n = w - 128 * half
        for h in (0, 1):
            m = half == h
            y[:, g[m]] = yk[h * 64:(h + 1) * 64][:, n[m]]
    return y


def kernel(**inputs):
    from concourse.bass_utils import run_bass_kernel_spmd
    if "prog" not in _PROGRAM_CACHE:
        _PROGRAM_CACHE["prog"] = build_program()
    nc = _PROGRAM_CACHE["prog"]
    in_maps, starts = host_prep(**{k: np.asarray(v) for k, v in inputs.items()})
    res = run_bass_kernel_spmd(nc, in_maps, core_ids=list(range(NCORES)))
    return assemble(res.results, starts)

